# revision 8
# baseline (speedup 1.0000x reference)
"""Trainium2 kernel for nn_Adaptive_Grid (retrieval_knn).

Pipeline:
  1. uniform grid sampling (voxelize + stable argsort + evenly spaced ranks)
     -> vertex table [8192, 3].  Exact integer/permutation computation, done
     host-side in numpy (bit-identical to the jax reference on CPU).
  2. 1-NN of every point against the 8192 vertices, on 8 NeuronCores:
     points sharded across cores, vertex table replicated.

Device kernel (SPMD, per core, Tile framework):
  score[p, v] = 2*p.v - |v|^2   (negated reference distance, bit-exact:
     products (2x)*vx == 2*(x*vx) and the K-dim accumulation order of the
     PE matmul matches the reference's left-to-right fp32 sum)
  computed as a K=4 matmul with stationary weights [2x, 2y, 2z, 1] per
  128-point block and moving operand [vx, vy, vz, -|v|^2].
  argmax with first-index tie-break (== jnp.argmin of distance) via the DVE
  max8 / max_index instructions over the [128, 8192] score rows.
"""

import numpy as np

N = 1048576
V = 8192
GRID = 64
NCORES = 8
NSHARD = N // NCORES          # 131072 points per core
P = 128                       # points per block (partition dim)
BLOCKS = NSHARD // P          # 1024
BLOCKS_PER_ITER = 8           # blocks per For_i iteration
OUTER = BLOCKS // BLOCKS_PER_ITER
VTILE = 512                   # verts per matmul (PSUM bank)
PSB = 2048                    # verts per psum tile (4 banks)


# ---------------------------------------------------------------------------
# stage 1: uniform grid sampling (host, exact)
# ---------------------------------------------------------------------------

def _uniform_grid_sampling(xyz: np.ndarray) -> np.ndarray:
    mins = xyz.min(axis=0)
    maxs = xyz.max(axis=0)
    extent = maxs - mins
    denom = np.where(extent > 0, extent, np.float32(1.0))
    unit = np.clip((xyz - mins) / denom, np.float32(0.0), np.float32(1.0 - 1e-6))
    vox = np.floor(unit * GRID).astype(np.int32)
    vid = (vox[:, 0] * GRID + vox[:, 1]) * GRID + vox[:, 2]
    order = np.argsort(vid, kind="stable")
    # jnp.linspace computes in float32 (jax default); replicate exactly:
    # floor(arange(V, f32) * (f32(N-1) / f32(V-1)))
    step = np.float32(N - 1) / np.float32(V - 1)
    sel = np.floor((np.arange(V, dtype=np.float32) * step).astype(np.float32)).astype(
        np.int32
    )
    return order[sel]


# ---------------------------------------------------------------------------
# workaround: this container's neuronxcc only accepts one sync-wait command
# per instruction; Tile's kernel-tail drain carries one wait per outstanding
# semaphore.  Split them across multiple drain instructions.
# ---------------------------------------------------------------------------

def _patch_tile_drain():
    import bass_rust as br
    import concourse.tile as tile_mod

    if getattr(tile_mod.TileContext, "_drain_patched", False):
        return

    def _drain_and_barrier(self, tick_clock, wait_clock):
        drain_inst = self.nc.sync.drain()
        wait_clock.add_sem_waits(
            drain_inst.ins, br.ScopedClock({None: tick_clock.global_clock})
        )
        si = drain_inst.ins.sync_info
        if si is not None:
            waits = list(si.on_wait or [])
            if len(waits) > 1:
                drain_inst.ins.sync_info = br.SyncInfo(
                    on_wait=waits[:1], on_update=list(si.on_update or [])
                )
                for i in range(1, len(waits)):
                    d2 = self.nc.sync.drain()
                    d2.ins.sync_info = br.SyncInfo(on_wait=waits[i : i + 1], on_update=[])
        self.nc.all_engine_barrier()
        popped = self.nc._tile_sem_poison_stack.pop()
        assert popped is self._sem_poison
        self.nc.clear_and_free_semaphores(list(self.sems.allocated().values()))
        self.nc.all_engine_barrier()

    tile_mod.TileContext._drain_and_barrier = _drain_and_barrier
    tile_mod.TileContext._drain_patched = True


def _split_multiwait_ctrl(nc):
    """Same neuronxcc limitation for drains emitted inside loop back-edges:
    split any Drain/Nop carrying >1 sync waits into a chain of single-wait
    drains (engine executes in order, waits are cumulative thresholds)."""
    import bass_rust as br
    import concourse.mybir as mybir

    cnt = 0
    for func in nc.m.functions:
        for bb in func.blocks:
            out = []
            changed = False
            for ins in bb.instructions:
                si = ins.sync_info
                waits = list(si.on_wait or []) if si is not None else []
                if len(waits) > 1:
                    # keep register-valued waits on the instruction itself
                    imm = [w for w in waits if w.wait_reg is None]
                    reg = [w for w in waits if w.wait_reg is not None]
                    keep = reg + imm[len(imm) - max(0, 1 - len(reg)) :]
                    hoist = imm[: len(imm) - max(0, 1 - len(reg))]
                    for w in hoist:
                        cnt += 1
                        d = mybir.InstNoOp(name=f"I-wsplit-{cnt}", ins=[], outs=[])
                        d.engine = ins.engine
                        d.sync_info = br.SyncInfo(on_wait=[w], on_update=[])
                        out.append(d)
                        changed = True
                    ins.sync_info = br.SyncInfo(
                        on_wait=keep, on_update=list(si.on_update or [])
                    )
                out.append(ins)
            if changed:
                bb.instructions = out
    return nc


# ---------------------------------------------------------------------------
# stage 2: device KNN graph
# ---------------------------------------------------------------------------

_GRAPH_CACHE = {}


def _build_knn_graph(nshard=NSHARD, v=V, blocks_per_iter=BLOCKS_PER_ITER):
    import concourse.bass as bass
    import concourse.mybir as mybir
    from concourse.tile import TileContext

    _patch_tile_drain()

    blocks = nshard // P
    outer = blocks // blocks_per_iter
    cpts = blocks_per_iter * P          # points per outer iteration
    n_ps = v // PSB                     # psum tiles per block
    n_mm = PSB // VTILE                 # matmuls per psum tile

    nc = bass.Bass()
    w4 = nc.declare_dram_parameter("w4", [4, nshard], mybir.dt.float32, isOutput=False)
    vr = nc.declare_dram_parameter("vr", [4, v], mybir.dt.float32, isOutput=False)
    out = nc.declare_dram_parameter("out", [P, blocks], mybir.dt.uint32, isOutput=True)

    with TileContext(nc) as tc:
        with (
            tc.tile_pool(name="verts", bufs=1) as vpool,
            tc.tile_pool(name="wts", bufs=2) as wpool,
            tc.tile_pool(name="score", bufs=2) as spool,
            tc.tile_pool(name="small", bufs=2) as mpool,
            tc.tile_pool(name="outp", bufs=2) as opool,
            tc.tile_pool(name="ps", bufs=2, space="PSUM") as pspool,
        ):
            vrt = vpool.tile([4, v], mybir.dt.float32)
            nc.sync.dma_start(vrt[:], vr[:])

            with tc.For_i(0, outer, 1) as c:
                wchunk = wpool.tile([4, cpts], mybir.dt.float32, tag="wchunk")
                nc.sync.dma_start(wchunk[:], w4[:, bass.ts(c, cpts)])
                outt = opool.tile([P, blocks_per_iter], mybir.dt.uint32, tag="outt")

                for j in range(blocks_per_iter):
                    wblk = wchunk[:, j * P : (j + 1) * P]
                    dbuf = spool.tile([P, v], mybir.dt.float32, tag="dbuf")
                    for t in range(n_ps):
                        ps = pspool.tile([P, PSB], mybir.dt.float32, tag="ps")
                        for s in range(n_mm):
                            v0 = t * PSB + s * VTILE
                            nc.tensor.matmul(
                                ps[:, s * VTILE : (s + 1) * VTILE],
                                lhsT=wblk,
                                rhs=vrt[:, v0 : v0 + VTILE],
                                start=True,
                                stop=True,
                            )
                        # PSUM -> SBUF copy on the scalar engine
                        nc.scalar.activation(
                            dbuf[:, t * PSB : (t + 1) * PSB],
                            ps[:],
                            mybir.ActivationFunctionType.Copy,
                        )
                    m8 = mpool.tile([P, 8], mybir.dt.float32, tag="m8")
                    i8 = mpool.tile([P, 8], mybir.dt.uint32, tag="i8")
                    nc.vector.max(m8[:], dbuf[:])
                    nc.vector.max_index(i8[:], m8[:], dbuf[:])
                    nc.vector.tensor_copy(outt[:, j : j + 1], i8[:, 0:1])

                nc.sync.dma_start(out[:, bass.ts(c, blocks_per_iter)], outt[:])

    _split_multiwait_ctrl(nc)
    return nc


def _get_graph(key="main", **kw):
    if key not in _GRAPH_CACHE:
        _GRAPH_CACHE[key] = _build_knn_graph(**kw)
    return _GRAPH_CACHE[key]


# ---------------------------------------------------------------------------
# host wrapper
# ---------------------------------------------------------------------------

def _prep_inputs(xyz: np.ndarray, vertex: np.ndarray):
    """Build per-core input maps: w4 shards + replicated vr."""
    x = xyz[:, 0].astype(np.float32)
    y = xyz[:, 1].astype(np.float32)
    z = xyz[:, 2].astype(np.float32)
    w4_full = np.stack(
        [2.0 * x, 2.0 * y, 2.0 * z, np.ones(N, np.float32)], axis=0
    ).astype(np.float32)

    vx = vertex[:, 0].astype(np.float32)
    vy = vertex[:, 1].astype(np.float32)
    vz = vertex[:, 2].astype(np.float32)
    vn = ((vx * vx + vy * vy) + vz * vz).astype(np.float32)
    vr = np.stack([vx, vy, vz, -vn], axis=0).astype(np.float32)

    in_maps = []
    for cid in range(NCORES):
        sl = slice(cid * NSHARD, (cid + 1) * NSHARD)
        in_maps.append({"w4": np.ascontiguousarray(w4_full[:, sl]), "vr": vr})
    return in_maps


LAST_RESULT = None  # BassKernelResults of the most recent kernel() call


def _enable_tracing():
    """Register the NTFF profile hook (missing antenv.axon_hooks shim) and
    stub the artifact upload.  Only needed when KNN_TRACE=1."""
    import sys, types

    if "antenv.axon_hooks" not in sys.modules:
        m = types.ModuleType("antenv.axon_hooks")
        m._hook = None
        m.set_axon_ntff_profile_hook = lambda h: setattr(m, "_hook", h)
        m.get_axon_ntff_profile_hook = lambda: m._hook
        sys.modules["antenv.axon_hooks"] = m
    from trn_agent_boot.trn_boot import _ntff_profile_via_ctypes

    sys.modules["antenv.axon_hooks"].set_axon_ntff_profile_hook(
        _ntff_profile_via_ctypes("/opt/axon/libaxon_pjrt.so")
    )
    import concourse.bass_utils as bu

    bu.upload_artifacts = lambda tmpdir: "local://" + tmpdir


def kernel(xyz: np.ndarray):
    import os
    from concourse.bass_utils import run_bass_kernel_spmd

    global LAST_RESULT
    xyz = np.asarray(xyz, dtype=np.float32)
    vidx = _uniform_grid_sampling(xyz)
    vertex = xyz[vidx]

    nc = _get_graph()
    in_maps = _prep_inputs(xyz, vertex)
    trace = os.environ.get("KNN_TRACE", "0") == "1"
    if trace:
        _enable_tracing()
    res = run_bass_kernel_spmd(
        nc, in_maps, core_ids=list(range(NCORES)), trace=trace
    )
    LAST_RESULT = res

    p2v = np.empty(N, dtype=np.int32)
    for cid in range(NCORES):
        o = res.results[cid]["out"]            # [P, blocks], point (b, p) at [p, b]
        p2v[cid * NSHARD : (cid + 1) * NSHARD] = o.T.reshape(-1).astype(np.int32)
    return vertex, p2v


# revision 21
# speedup vs baseline: 1.5088x; 1.5088x over previous
"""Trainium2 kernel for nn_Adaptive_Grid (retrieval_knn).

Pipeline:
  1. uniform grid sampling (voxelize + stable argsort + evenly spaced ranks)
     -> vertex table [8192, 3].  Exact integer/permutation computation, done
     host-side in numpy (bit-identical to the jax reference on CPU).
  2. 1-NN of every point against the 8192 vertices, on 8 NeuronCores:
     points sharded across cores, vertex table replicated.

Device kernel (SPMD, per core, Tile framework):
  score[p, v] = 2*p.v - |v|^2   (negated reference distance)
  computed as a K=21 bf16 matmul: each fp32 operand is split into three
  bf16 terms (x = xh + xm + xl, ~24 mantissa bits); keeping product pairs
  down to 2^-16 significance reproduces the fp32 score to ~1-2 ulp.  bf16
  streams the PE at 1 cycle/column (vs 4 for fp32), and K<=128 does not
  change matmul time.  Stationary weights = the 21 point-side rows per
  128-point block; moving operand = the 21 vert-side rows.
  argmax with first-index tie-break (== jnp.argmin of distance) via the DVE
  max8 / max_index instructions over the [128, 8192] score rows.
"""

import numpy as np

N = 1048576
V = 8192
GRID = 64
NCORES = 8
NSHARD = N // NCORES          # 131072 points per core
P = 128                       # points per block (partition dim)
BLOCKS = NSHARD // P          # 1024
BLOCKS_PER_ITER = 8           # blocks per For_i iteration
OUTER = BLOCKS // BLOCKS_PER_ITER
VTILE = 512                   # verts per matmul (PSUM bank)
PSB = 2048                    # verts per psum tile (4 banks)
KROWS = 21                    # bf16-split contraction rows


# ---------------------------------------------------------------------------
# stage 1: uniform grid sampling (host, exact)
# ---------------------------------------------------------------------------

def _uniform_grid_sampling(xyz: np.ndarray) -> np.ndarray:
    mins = xyz.min(axis=0)
    maxs = xyz.max(axis=0)
    extent = maxs - mins
    denom = np.where(extent > 0, extent, np.float32(1.0))
    unit = np.clip((xyz - mins) / denom, np.float32(0.0), np.float32(1.0 - 1e-6))
    vox = np.floor(unit * GRID).astype(np.int32)
    vid = (vox[:, 0] * GRID + vox[:, 1]) * GRID + vox[:, 2]
    order = np.argsort(vid, kind="stable")
    # jnp.linspace computes in float32 (jax default); replicate exactly:
    # floor(arange(V, f32) * (f32(N-1) / f32(V-1)))
    step = np.float32(N - 1) / np.float32(V - 1)
    sel = np.floor((np.arange(V, dtype=np.float32) * step).astype(np.float32)).astype(
        np.int32
    )
    return order[sel]


# ---------------------------------------------------------------------------
# workaround: this container's neuronxcc only accepts one sync-wait command
# per instruction; Tile's kernel-tail drain carries one wait per outstanding
# semaphore.  Split them across multiple drain instructions.
# ---------------------------------------------------------------------------

def _patch_tile_drain():
    import bass_rust as br
    import concourse.tile as tile_mod

    if getattr(tile_mod.TileContext, "_drain_patched", False):
        return

    def _drain_and_barrier(self, tick_clock, wait_clock):
        drain_inst = self.nc.sync.drain()
        wait_clock.add_sem_waits(
            drain_inst.ins, br.ScopedClock({None: tick_clock.global_clock})
        )
        si = drain_inst.ins.sync_info
        if si is not None:
            waits = list(si.on_wait or [])
            if len(waits) > 1:
                drain_inst.ins.sync_info = br.SyncInfo(
                    on_wait=waits[:1], on_update=list(si.on_update or [])
                )
                for i in range(1, len(waits)):
                    d2 = self.nc.sync.drain()
                    d2.ins.sync_info = br.SyncInfo(on_wait=waits[i : i + 1], on_update=[])
        self.nc.all_engine_barrier()
        popped = self.nc._tile_sem_poison_stack.pop()
        assert popped is self._sem_poison
        self.nc.clear_and_free_semaphores(list(self.sems.allocated().values()))
        self.nc.all_engine_barrier()

    tile_mod.TileContext._drain_and_barrier = _drain_and_barrier
    tile_mod.TileContext._drain_patched = True


def _split_multiwait_ctrl(nc):
    """Same neuronxcc limitation for drains emitted inside loop back-edges:
    split any Drain/Nop carrying >1 sync waits into a chain of single-wait
    drains (engine executes in order, waits are cumulative thresholds)."""
    import bass_rust as br
    import concourse.mybir as mybir

    cnt = 0
    for func in nc.m.functions:
        for bb in func.blocks:
            out = []
            changed = False
            for ins in bb.instructions:
                si = ins.sync_info
                waits = list(si.on_wait or []) if si is not None else []
                if len(waits) > 1:
                    # keep register-valued waits on the instruction itself
                    imm = [w for w in waits if w.wait_reg is None]
                    reg = [w for w in waits if w.wait_reg is not None]
                    keep = reg + imm[len(imm) - max(0, 1 - len(reg)) :]
                    hoist = imm[: len(imm) - max(0, 1 - len(reg))]
                    for w in hoist:
                        cnt += 1
                        d = mybir.InstNoOp(name=f"I-wsplit-{cnt}", ins=[], outs=[])
                        d.engine = ins.engine
                        d.sync_info = br.SyncInfo(on_wait=[w], on_update=[])
                        out.append(d)
                        changed = True
                    ins.sync_info = br.SyncInfo(
                        on_wait=keep, on_update=list(si.on_update or [])
                    )
                out.append(ins)
            if changed:
                bb.instructions = out
    return nc


# ---------------------------------------------------------------------------
# stage 2: device KNN graph
# ---------------------------------------------------------------------------

_GRAPH_CACHE = {}


def _build_knn_graph(nshard=NSHARD, v=V, blocks_per_iter=BLOCKS_PER_ITER):
    import concourse.bass as bass
    import concourse.mybir as mybir
    from concourse.tile import TileContext

    _patch_tile_drain()

    blocks = nshard // P
    outer = blocks // blocks_per_iter
    cpts = blocks_per_iter * P          # points per outer iteration
    n_ps = v // PSB                     # psum tiles per block
    n_mm = PSB // VTILE                 # matmuls per psum tile

    nc = bass.Bass()
    w4 = nc.declare_dram_parameter(
        "w4", [KROWS, nshard], mybir.dt.bfloat16, isOutput=False
    )
    vr = nc.declare_dram_parameter("vr", [KROWS, v], mybir.dt.bfloat16, isOutput=False)
    out = nc.declare_dram_parameter("out", [P, blocks], mybir.dt.uint32, isOutput=True)

    with TileContext(nc) as tc:
        with (
            tc.tile_pool(name="verts", bufs=1) as vpool,
            tc.tile_pool(name="wts", bufs=2) as wpool,
            tc.tile_pool(name="score", bufs=2) as spool,
            tc.tile_pool(name="small", bufs=2) as mpool,
            tc.tile_pool(name="outp", bufs=2) as opool,
            tc.tile_pool(name="ps", bufs=2, space="PSUM") as pspool,
        ):
            vrt = vpool.tile([KROWS, v], mybir.dt.bfloat16)
            nc.sync.dma_start(vrt[:], vr[:])

            with tc.For_i(0, outer, 1) as c:
                wchunk = wpool.tile([KROWS, cpts], mybir.dt.bfloat16, tag="wchunk")
                nc.sync.dma_start(wchunk[:], w4[:, bass.ts(c, cpts)])
                outt = opool.tile([P, 8 * blocks_per_iter], mybir.dt.uint32, tag="outt")

                for j in range(blocks_per_iter):
                    wblk = wchunk[:, j * P : (j + 1) * P]
                    dbuf = spool.tile([P, v], mybir.dt.float32, tag="dbuf")
                    for t in range(n_ps):
                        ps = pspool.tile([P, PSB], mybir.dt.float32, tag="ps")
                        for s in range(n_mm):
                            v0 = t * PSB + s * VTILE
                            nc.tensor.matmul(
                                ps[:, s * VTILE : (s + 1) * VTILE],
                                lhsT=wblk,
                                rhs=vrt[:, v0 : v0 + VTILE],
                                start=True,
                                stop=True,
                            )
                        # PSUM -> SBUF copy on the scalar engine
                        nc.scalar.activation(
                            dbuf[:, t * PSB : (t + 1) * PSB],
                            ps[:],
                            mybir.ActivationFunctionType.Copy,
                        )
                    m8 = mpool.tile([P, 8], mybir.dt.float32, tag="m8")
                    nc.vector.max(m8[:], dbuf[:])
                    nc.vector.max_index(
                        outt[:, j * 8 : (j + 1) * 8], m8[:], dbuf[:]
                    )

                # column 0 of each block's 8-wide max_index result
                nc.sync.dma_start(
                    out[:, bass.ts(c, blocks_per_iter)],
                    outt.rearrange("p (b e) -> p b e", e=8)[:, :, 0],
                )

    _split_multiwait_ctrl(nc)
    return nc


def _get_graph(key="main", **kw):
    if key not in _GRAPH_CACHE:
        _GRAPH_CACHE[key] = _build_knn_graph(**kw)
    return _GRAPH_CACHE[key]


# ---------------------------------------------------------------------------
# stage 2 variant B: exp-selection kernel (no DVE max_index pass)
#
# Pass A (points on partitions): K=21 bf16 matmul -> scores; DVE reduce_max
#   -> m[p] per point.  m is then split into 4 bf16 terms (exact: 4x8 >= 24
#   mantissa bits) and transposed into 4 extra moving rows.
# Pass B (verts on partitions): K=25 bf16 matmul -> z[v,p] = score - m.
#   Each subtraction step is Sterbenz-exact near the max, so z <= 0 with
#   z == 0 iff score == m.  ACT computes em = exp(1e10 * z) in {1, 0}.
# Extract: tiny PE matmul with weights [hi(v), lo(v), 1] accumulated over
#   vert blocks -> [3, pts] = (hi*64+lo == index, tie count).  DVE divides,
#   rounds and emits int indices.  Exact except exact-fp32 score ties
#   (O(10) points per 1M, averaged).
# ---------------------------------------------------------------------------

KB = KROWS + 4                 # pass-B contraction rows (21 + 4 m-split rows)
PTS_ITER = 1024                # points per For_i iteration (v2b)
EXPK = 1.0e10


def _build_knn_graph_v2b(nshard=NSHARD, v=V):
    import concourse.bass as bass
    import concourse.mybir as mybir
    from concourse.tile import TileContext

    _patch_tile_drain()

    iters = nshard // PTS_ITER
    blocks_per_iter = PTS_ITER // P          # 8
    n_vb = v // P                            # 64 vert blocks
    n_pa = v // 1024                         # pass-A psum tiles per block

    nc = bass.Bass()
    w4 = nc.declare_dram_parameter(
        "w4", [KROWS, nshard], mybir.dt.bfloat16, isOutput=False
    )
    vr = nc.declare_dram_parameter("vr", [KROWS, v], mybir.dt.bfloat16, isOutput=False)
    vb = nc.declare_dram_parameter("vb", [KB, v], mybir.dt.bfloat16, isOutput=False)
    wx = nc.declare_dram_parameter(
        "wx", [P, 3 * n_vb], mybir.dt.bfloat16, isOutput=False
    )
    ident = nc.declare_dram_parameter(
        "ident", [P, P], mybir.dt.bfloat16, isOutput=False
    )
    out = nc.declare_dram_parameter("out", [1, nshard], mybir.dt.uint32, isOutput=True)

    F32 = mybir.dt.float32
    BF = mybir.dt.bfloat16

    with TileContext(nc) as tc:
        with (
            tc.tile_pool(name="const", bufs=1) as cpool,
            tc.tile_pool(name="wb", bufs=2) as wbpool,
            tc.tile_pool(name="em", bufs=2) as empool,
            tc.tile_pool(name="small", bufs=2) as mpool,
            tc.tile_pool(name="fin", bufs=2) as fpool,
            tc.tile_pool(name="psA", bufs=2, space="PSUM") as psa,
            tc.tile_pool(name="psB", bufs=1, space="PSUM") as psb,
            tc.tile_pool(name="acc", bufs=1, space="PSUM") as pacc,
        ):
            vrt = cpool.tile([KROWS, v], BF)
            nc.sync.dma_start(vrt[:], vr[:])
            vbt = cpool.tile([KB, v], BF)
            nc.sync.dma_start(vbt[:], vb[:])
            wxt = cpool.tile([P, 3 * n_vb], BF)
            nc.sync.dma_start(wxt[:], wx[:])
            idt = cpool.tile([P, P], BF)
            nc.sync.dma_start(idt[:], ident[:])

            with tc.For_i(0, iters, 1, hint_engines=(mybir.EngineType.PE,)) as c:
                wb_t = wbpool.tile([KB, PTS_ITER], BF, tag="wb")
                nc.sync.dma_start(wb_t[0:KROWS, :], w4[:, bass.ts(c, PTS_ITER)])

                # ---- pass A: per-point max score ----
                mblk = mpool.tile([P, blocks_per_iter], F32, tag="mblk")
                for j in range(blocks_per_iter):
                    lhs = wb_t[0:KROWS, j * P : (j + 1) * P]
                    m16 = mpool.tile([P, n_pa], F32, tag="m16")
                    for t in range(n_pa):
                        pa = psa.tile([P, 1024], F32, tag="pa")
                        for s in range(2):
                            v0 = t * 1024 + s * VTILE
                            nc.tensor.matmul(
                                pa[:, s * VTILE : (s + 1) * VTILE],
                                lhsT=lhs,
                                rhs=vrt[:, v0 : v0 + VTILE],
                                start=True,
                                stop=True,
                            )
                        nc.vector.tensor_reduce(
                            m16[:, t : t + 1],
                            pa[:],
                            axis=mybir.AxisListType.X,
                            op=mybir.AluOpType.max,
                        )
                    nc.vector.tensor_reduce(
                        mblk[:, j : j + 1],
                        m16[:],
                        axis=mybir.AxisListType.X,
                        op=mybir.AluOpType.max,
                    )

                # ---- split -m into 4 exact bf16 terms and transpose ----
                negm = mpool.tile([P, blocks_per_iter], F32, tag="negm")
                nc.vector.tensor_scalar_mul(negm[:], mblk[:], -1.0)
                resid = negm
                for i in range(4):
                    sp_bf = mpool.tile([P, blocks_per_iter], BF, tag=f"spbf{i}")
                    nc.vector.tensor_copy(sp_bf[:], resid[:])
                    if i < 3:
                        sp_f = mpool.tile([P, blocks_per_iter], F32, tag=f"spf{i}")
                        nc.vector.tensor_copy(sp_f[:], sp_bf[:])
                        nresid = mpool.tile(
                            [P, blocks_per_iter], F32, tag=f"resid{i}"
                        )
                        nc.vector.tensor_tensor(
                            out=nresid[:],
                            in0=resid[:],
                            in1=sp_f[:],
                            op=mybir.AluOpType.subtract,
                        )
                        resid = nresid
                    # transpose [P, 8] -> [8, P] and place as row KROWS+i
                    # (shares the pass-A PSUM pool slots)
                    tps = psa.tile([blocks_per_iter, P], BF, tag="pa")
                    nc.tensor.transpose(tps[:], sp_bf[:], idt[:])
                    nc.sync.dma_start(
                        wb_t[KROWS + i : KROWS + i + 1, :].rearrange(
                            "a (j p) -> a j p", j=blocks_per_iter
                        ),
                        tps[:],
                    )

                # ---- pass B + extract ----
                acc = pacc.tile([3, PTS_ITER], F32, tag="acc")
                for g in range(n_vb):
                    pb = psb.tile([P, PTS_ITER], F32, tag="pb")
                    for s in range(2):
                        nc.tensor.matmul(
                            pb[:, s * VTILE : (s + 1) * VTILE],
                            lhsT=vbt[:, g * P : (g + 1) * P],
                            rhs=wb_t[:, s * VTILE : (s + 1) * VTILE],
                            start=True,
                            stop=True,
                        )
                    em = empool.tile([P, PTS_ITER], BF, tag="em")
                    nc.scalar.activation(
                        em[:], pb[:], mybir.ActivationFunctionType.Exp, scale=EXPK
                    )
                    for s in range(2):
                        nc.tensor.matmul(
                            acc[:, s * VTILE : (s + 1) * VTILE],
                            lhsT=wxt[:, 3 * g : 3 * g + 3],
                            rhs=em[:, s * VTILE : (s + 1) * VTILE],
                            start=(g == 0),
                            stop=(g == n_vb - 1),
                            skip_group_check=True,
                        )

                # ---- finalize: idx = round(accH/cnt)*64 + round(accL/cnt) ----
                rec = fpool.tile([1, PTS_ITER], F32, tag="rec")
                nc.vector.reciprocal(rec[:], acc[2:3, :])
                hif = fpool.tile([1, PTS_ITER], F32, tag="hif")
                nc.vector.tensor_tensor(
                    out=hif[:], in0=acc[0:1, :], in1=rec[:], op=mybir.AluOpType.mult
                )
                lof = fpool.tile([1, PTS_ITER], F32, tag="lof")
                nc.vector.tensor_tensor(
                    out=lof[:], in0=acc[1:2, :], in1=rec[:], op=mybir.AluOpType.mult
                )
                idxf = fpool.tile([1, PTS_ITER], F32, tag="idxf")
                nc.vector.tensor_scalar(
                    out=idxf[:],
                    in0=hif[:],
                    scalar1=64.0,
                    scalar2=None,
                    op0=mybir.AluOpType.mult,
                )
                nc.vector.tensor_tensor(
                    out=idxf[:], in0=idxf[:], in1=lof[:], op=mybir.AluOpType.add
                )
                nc.vector.tensor_scalar_add(idxf[:], idxf[:], 0.25)
                idxu = fpool.tile([1, PTS_ITER], mybir.dt.uint32, tag="idxu")
                nc.vector.tensor_copy(idxu[:], idxf[:])
                nc.sync.dma_start(out[:, bass.ts(c, PTS_ITER)], idxu[:])

    _split_multiwait_ctrl(nc)
    return nc


def _prep_inputs_v2b(xyz: np.ndarray, vertex: np.ndarray):
    import ml_dtypes

    bf = ml_dtypes.bfloat16
    w21, vr21 = _split_rows(xyz, vertex)
    vcnt = vertex.shape[0]
    vb = np.concatenate([vr21, np.ones((4, vcnt), bf)], axis=0).astype(bf)

    n_vb = vcnt // P
    vv = np.arange(vcnt, dtype=np.int64)
    hi = (vv >> 6).astype(np.float32)
    lo = (vv & 63).astype(np.float32)
    wx = np.zeros((P, 3 * n_vb), np.float32)
    for g in range(n_vb):
        wx[:, 3 * g] = hi[g * P : (g + 1) * P]
        wx[:, 3 * g + 1] = lo[g * P : (g + 1) * P]
        wx[:, 3 * g + 2] = 1.0
    wx = wx.astype(bf)
    ident = np.eye(P, dtype=bf)

    nshard = xyz.shape[0] // NCORES
    in_maps = []
    for cid in range(NCORES):
        sl = slice(cid * nshard, (cid + 1) * nshard)
        in_maps.append(
            {
                "w4": np.ascontiguousarray(w21[:, sl]),
                "vr": vr21,
                "vb": vb,
                "wx": wx,
                "ident": ident,
            }
        )
    return in_maps


# ---------------------------------------------------------------------------
# host wrapper
# ---------------------------------------------------------------------------

def _bf16_split3(a: np.ndarray):
    """fp32 -> (hi, mid, lo) bf16 triplet with hi+mid+lo ~ a to ~26 bits."""
    import ml_dtypes

    bf = ml_dtypes.bfloat16
    h = a.astype(bf)
    r1 = (a - h.astype(np.float32)).astype(np.float32)
    m = r1.astype(bf)
    r2 = (r1 - m.astype(np.float32)).astype(np.float32)
    l = r2.astype(bf)
    return h, m, l


def _split_rows(pts: np.ndarray, vertex: np.ndarray):
    """Build the K=21 bf16 row pair tables.

    Returns (w21 [21, n] bf16 point-side rows, vr21 [21, V] bf16 vert-side
    rows).  Row k contributes w21[k, p] * vr21[k, v] to score[p, v]; pairs
    are ordered by decreasing significance so the fp32 PSUM accumulation
    rounds like a big-to-small sum.
    """
    import ml_dtypes

    bf = ml_dtypes.bfloat16
    n = pts.shape[0]
    vcnt = vertex.shape[0]
    ones = np.ones(n, bf)

    psplit = [_bf16_split3((2.0 * pts[:, c]).astype(np.float32)) for c in range(3)]
    vsplit = [_bf16_split3(vertex[:, c].astype(np.float32)) for c in range(3)]
    vx, vy, vz = (vertex[:, c].astype(np.float32) for c in range(3))
    vn = ((vx * vx + vy * vy) + vz * vz).astype(np.float32)
    wh, wm, wl = _bf16_split3(-vn)

    wrows, vrows = [], []

    def pair(pr, vrow):
        wrows.append(pr)
        vrows.append(vrow)

    # level 0: ~1
    for c in range(3):
        pair(psplit[c][0], vsplit[c][0])
    pair(ones, wh)
    # level 1: ~2^-8
    for c in range(3):
        pair(psplit[c][0], vsplit[c][1])
        pair(psplit[c][1], vsplit[c][0])
    pair(ones, wm)
    # level 2: ~2^-16
    for c in range(3):
        pair(psplit[c][0], vsplit[c][2])
        pair(psplit[c][1], vsplit[c][1])
        pair(psplit[c][2], vsplit[c][0])
    pair(ones, wl)

    w21 = np.stack(wrows, axis=0).astype(bf)
    vr21 = np.stack(vrows, axis=0).astype(bf)
    assert w21.shape[0] == KROWS
    return w21, vr21


def _prep_inputs(xyz: np.ndarray, vertex: np.ndarray):
    """Build per-core input maps: w21 shards + replicated vr21."""
    w21, vr21 = _split_rows(xyz, vertex)
    in_maps = []
    for cid in range(NCORES):
        sl = slice(cid * NSHARD, (cid + 1) * NSHARD)
        in_maps.append({"w4": np.ascontiguousarray(w21[:, sl]), "vr": vr21})
    return in_maps


LAST_RESULT = None  # BassKernelResults of the most recent kernel() call


def _enable_tracing():
    """Register the NTFF profile hook (missing antenv.axon_hooks shim) and
    stub the artifact upload.  Only needed when KNN_TRACE=1."""
    import sys, types

    if "antenv.axon_hooks" not in sys.modules:
        m = types.ModuleType("antenv.axon_hooks")
        m._hook = None
        m.set_axon_ntff_profile_hook = lambda h: setattr(m, "_hook", h)
        m.get_axon_ntff_profile_hook = lambda: m._hook
        sys.modules["antenv.axon_hooks"] = m
    from trn_agent_boot.trn_boot import _ntff_profile_via_ctypes

    sys.modules["antenv.axon_hooks"].set_axon_ntff_profile_hook(
        _ntff_profile_via_ctypes("/opt/axon/libaxon_pjrt.so")
    )
    import concourse.bass_utils as bu

    bu.upload_artifacts = lambda tmpdir: "local://" + tmpdir


USE_V2B = True  # exp-selection kernel (faster); False = max_index kernel


def kernel(xyz: np.ndarray):
    import os
    from concourse.bass_utils import run_bass_kernel_spmd

    global LAST_RESULT
    xyz = np.asarray(xyz, dtype=np.float32)
    vidx = _uniform_grid_sampling(xyz)
    vertex = xyz[vidx]

    use_v2b = USE_V2B if os.environ.get("KNN_V2B") is None else (
        os.environ["KNN_V2B"] == "1"
    )
    trace = os.environ.get("KNN_TRACE", "0") == "1"
    if trace:
        _enable_tracing()

    if use_v2b:
        if "v2b" not in _GRAPH_CACHE:
            _GRAPH_CACHE["v2b"] = _build_knn_graph_v2b()
        nc = _GRAPH_CACHE["v2b"]
        in_maps = _prep_inputs_v2b(xyz, vertex)
    else:
        nc = _get_graph()
        in_maps = _prep_inputs(xyz, vertex)

    res = run_bass_kernel_spmd(
        nc, in_maps, core_ids=list(range(NCORES)), trace=trace
    )
    LAST_RESULT = res

    p2v = np.empty(N, dtype=np.int32)
    for cid in range(NCORES):
        o = res.results[cid]["out"]
        if use_v2b:
            p2v[cid * NSHARD : (cid + 1) * NSHARD] = o.reshape(-1).astype(np.int32)
        else:
            # [P, blocks], point (b, p) at [p, b]
            p2v[cid * NSHARD : (cid + 1) * NSHARD] = o.T.reshape(-1).astype(np.int32)
    return vertex, p2v
